# revision 64
# baseline (speedup 1.0000x reference)
"""Trainium2 Bass kernel for CausalMessagePassing (B=8, N=2048, D=256, H=4).

Strategy: data-parallel across 8 NeuronCores, one graph per core.
Per-core dataflow is column-major ("transposed spine"):
  x -> x^T (PE transpose); q^T,k^T col-major; v row-major with a ones
  column appended per head (yields softmax sums for free).
  scores^T[j,i] = k_h^T.T @ q_h^T per head, in float32r (1 cyc/row,
  ~tf32 precision). Causal mask applied on-chip via affine_select on the
  diagonal tiles only; fully-masked i<j tiles are never computed (the
  [N,N] mask input is tril(ones) by construction and is never DMA'd).
  e = exp(scores/sqrt(hd)) on ACT, psum->sbuf.
  ctx'^T[65,i] = v'.T @ e^T accumulated per 512-col quarter in PSUM;
  row 64 = softmax sums. Each quarter is normalized as soon as its
  accumulation finishes: reciprocal (DVE) -> partition_broadcast (Pool)
  -> multiply fused into the PSUM eviction (DVE).
  messages^T = Wo.T @ ectx^T (+bo); u^T = relu(Wu.T @ [x^T; m^T] + bu);
  PE-transpose u^T -> u and DMA out.
  Extras: PE HAM warm-up + ACT exp-table preload during the input DMA
  window; all DMAs batched; phases interleaved for engine overlap.
"""
import sys

sys.path.insert(0, "/opt/trn_rl_repo")

import numpy as np

import concourse.bass as bass  # noqa: F401
import concourse.mybir as mybir
import concourse.tile as tile
from concourse import bacc
from concourse.masks import make_identity

B, N, DM, H = 8, 2048, 256, 4
HD = DM // H  # 64
NB = N // 128  # 16 j-blocks
IT = N // 512  # 4 i-tiles
F32 = mybir.dt.float32
F32R = mybir.dt.float32r


def build_program():
    nc = bacc.Bacc("TRN2", target_bir_lowering=False, debug=False)
    x_d = nc.dram_tensor("x", [N, DM], F32, kind="ExternalInput").ap()
    wq_d = nc.dram_tensor("wq", [DM, DM], F32, kind="ExternalInput").ap()
    wk_d = nc.dram_tensor("wk", [DM, DM], F32, kind="ExternalInput").ap()
    wv_d = nc.dram_tensor("wv", [DM, DM], F32, kind="ExternalInput").ap()
    wo_d = nc.dram_tensor("wo", [DM, DM], F32, kind="ExternalInput").ap()
    wu_d = nc.dram_tensor("wu", [2 * DM, DM], F32, kind="ExternalInput").ap()
    bq_d = nc.dram_tensor("bq", [DM], F32, kind="ExternalInput").ap()
    bk_d = nc.dram_tensor("bk", [DM], F32, kind="ExternalInput").ap()
    bv_d = nc.dram_tensor("bv", [DM], F32, kind="ExternalInput").ap()
    bo_d = nc.dram_tensor("bo", [DM], F32, kind="ExternalInput").ap()
    bu_d = nc.dram_tensor("bu", [DM], F32, kind="ExternalInput").ap()
    out_d = nc.dram_tensor("out", [N, DM], F32, kind="ExternalOutput").ap()

    def r(ap):
        return ap.bitcast(F32R)

    with tile.TileContext(nc) as tc:
        with (
            tc.tile_pool(name="const", bufs=1) as cpool,
            tc.tile_pool(name="big", bufs=1) as bpool,
            tc.tile_pool(name="work", bufs=3) as wpool,
            tc.tile_pool(name="mm", bufs=2, space="PSUM") as mmp,
            tc.tile_pool(name="sc", bufs=4, space="PSUM") as scp,
            tc.tile_pool(name="ctxp", bufs=1, space="PSUM") as ctxp,
        )            :
            # ---- constants / weights (batched DMAs) ----
            ident = cpool.tile([128, 128], F32R, tag="ident")
            ident_f = cpool.tile([128, 128], F32, tag="identf")
            make_identity(nc, ident_f[:])
            nc.vector.tensor_copy(ident[:], ident_f[:])
            # PE HAM warm-up during the input-DMA window: dummy transposes
            # keep the PE busy so real matmuls start at full clock. Also
            # preload the ACT exp table set off the critical path.
            warm = scp.tile([128, 512], F32R, tag="sc", name="warm")
            for _ in range(32):
                nc.tensor.transpose(warm[0:128, 0:128], ident[:], ident[:])
            wexp = cpool.tile([1, 8], F32, tag="wexp")
            nc.scalar.activation(
                wexp[:], ident_f[0:1, 0:8], mybir.ActivationFunctionType.Exp
            )
            # each W loaded as one DMA: [128, 2*DM], chunk c at cols [c*DM, (c+1)*DM)
            wq_a = cpool.tile([128, 2 * DM], F32R, tag="wqa")
            wk_a = cpool.tile([128, 2 * DM], F32R, tag="wka")
            wv_a = cpool.tile([128, 2 * DM], F32R, tag="wva")
            wo_a = cpool.tile([128, 2 * DM], F32R, tag="woa")
            wu_a = cpool.tile([128, 4 * DM], F32R, tag="wua")

            def dma_w(t_sb, t_d):
                nc.sync.dma_start(
                    t_sb[:].rearrange("p (c d) -> p c d", d=DM),
                    r(t_d.rearrange("(c p) d -> p c d", p=128)),
                )

            stage = cpool.tile([128, NB * DM], F32R, tag="stage")
            xs_all = stage
            x_r = r(x_d.rearrange("(t p) d -> p t d", p=128))

            def dma_x(g):
                nc.sync.dma_start(
                    xs_all[:, g * 2 * DM:(g + 1) * 2 * DM].rearrange(
                        "p (t d) -> p t d", d=DM
                    ),
                    x_r[:, g * 2:(g + 1) * 2, :],
                )

            dma_x(0)
            dma_x(1)
            dma_w(wq_a, wq_d)
            dma_w(wk_a, wk_d)
            dma_x(2)
            dma_x(3)
            wq_sb = [wq_a[:, c * DM:(c + 1) * DM] for c in range(2)]
            wk_sb = [wk_a[:, c * DM:(c + 1) * DM] for c in range(2)]
            wv_sb = [wv_a[:, c * DM:(c + 1) * DM] for c in range(2)]
            wo_sb = [wo_a[:, c * DM:(c + 1) * DM] for c in range(2)]
            wu_sb = [wu_a[:, c * DM:(c + 1) * DM] for c in range(4)]
            bq_a = cpool.tile([128, 2], F32, tag="bqa")
            bk_a = cpool.tile([128, 2], F32, tag="bka")
            bo_a = cpool.tile([128, 2], F32, tag="boa")
            bu_a = cpool.tile([128, 2], F32, tag="bua")
            for t_sb, t_d in ((bq_a, bq_d), (bk_a, bk_d), (bo_a, bo_d), (bu_a, bu_d)):
                nc.sync.dma_start(t_sb[:], t_d.rearrange("(c p) -> p c", p=128))
            bq_c = [bq_a[:, b:b + 1] for b in range(2)]
            bk_c = [bk_a[:, b:b + 1] for b in range(2)]
            bo_c = [bo_a[:, b:b + 1] for b in range(2)]
            bu_c = [bu_a[:, b:b + 1] for b in range(2)]
            # bv broadcast tile [128, 256] (f32; only used by DVE add)
            bv_row = cpool.tile([1, DM], F32, tag="bvrow")
            nc.sync.dma_start(bv_row[:], bv_d.rearrange("(b a) -> b a", b=1))
            ones1 = cpool.tile([1, 128], F32, tag="ones1")
            nc.gpsimd.memset(ones1[:], 1.0)
            bv_bc = cpool.tile([128, DM], F32, tag="bvbc")
            pt = mmp.tile([128, DM], F32, tag="mm")
            nc.tensor.matmul(pt[:], ones1[:], bv_row[:], start=True, stop=True)
            nc.vector.tensor_copy(bv_bc[:], pt[:])
            ones_r = cpool.tile([1, 64], F32R, tag="onesr")
            ones_rf = cpool.tile([1, 64], F32, tag="onesrf")
            nc.gpsimd.memset(ones_rf[:], 1.0)
            nc.vector.tensor_copy(ones_r[:], ones_rf[:])
            ones_col4 = cpool.tile([128, 4], F32, tag="onescol4")
            nc.gpsimd.memset(ones_col4[:], 1.0)
            # ---- rest of x + remaining weights ----
            for g in range(4, 8):
                dma_x(g)
            dma_w(wv_a, wv_d)
            dma_w(wo_a, wo_d)
            dma_w(wu_a, wu_d)

            xT = [bpool.tile([128, N], F32R, tag=f"xT{c}", name=f"xT{c}") for c in range(2)]
            qT = [bpool.tile([128, N], F32R, tag=f"qT{b}", name=f"qT{b}") for b in range(2)]
            kT = [bpool.tile([128, N], F32R, tag=f"kT{b}", name=f"kT{b}") for b in range(2)]

            def emit_qk_it(blk, it):
                for w_sb, b_c, dstT in ((wq_sb, bq_c, qT), (wk_sb, bk_c, kT)):
                    pt = mmp.tile([128, 512], F32, tag="mm", name="qkpt")
                    for c in range(2):
                        nc.tensor.matmul(
                            pt[:],
                            w_sb[c][:, blk * 128:(blk + 1) * 128],
                            xT[c][:, it * 512:(it + 1) * 512],
                            start=(c == 0),
                            stop=(c == 1),
                        )
                    nc.vector.tensor_scalar_add(
                        dstT[blk][:, it * 512:(it + 1) * 512], pt[:], b_c[blk][:]
                    )

            def emit_qk(blk):
                for it in range(IT):
                    emit_qk_it(blk, it)

            # interleave x transposes with q/k(blk0) per i-tile so scores can
            # start after the first quarter of the transpose stream
            for it in range(IT):
                for ib in range(it * 4, (it + 1) * 4):
                    for c in range(2):
                        tp = mmp.tile([128, 128], F32R, tag="mm")
                        nc.tensor.transpose(
                            tp[:], xs_all[:, ib * DM + c * 128:ib * DM + (c + 1) * 128], ident[:]
                        )
                        nc.vector.tensor_copy(xT[c][:, ib * 128:(ib + 1) * 128], tp[:])
                emit_qk_it(0, it)

            # ---- v (row-major, with ones col per head) ----
            # v_sb[jb]: [128, 4*65]; head h data at cols 65h..65h+63, ones at 65h+64
            v_sb = [bpool.tile([128, 4 * 65], F32R, tag=f"v{jb}", name=f"v{jb}") for jb in range(NB)]

            def emit_v(jb):
                v4 = v_sb[jb][:].rearrange("p (h e) -> p h e", e=65)
                nc.vector.tensor_copy(
                    v4[:, :, 64:65],
                    ones_col4[:].rearrange("p (h e) -> p h e", e=1),
                )
                pt = mmp.tile([128, DM], F32, tag="mm", name="vpt")
                for c in range(2):
                    nc.tensor.matmul(
                        pt[:],
                        xT[c][:, jb * 128:(jb + 1) * 128],
                        wv_sb[c][:],
                        start=(c == 0),
                        stop=(c == 1),
                    )
                nc.vector.tensor_tensor(
                    v4[:, :, 0:64],
                    pt[:].rearrange("p (h e) -> p h e", e=64),
                    bv_bc[:].rearrange("p (h e) -> p h e", e=64),
                    op=mybir.AluOpType.add,
                )

            # ---- attention per head ----
            ectx = [bpool.tile([128, N], F32R, tag=f"ectx{b}", name=f"ectx{b}") for b in range(2)]
            m_sb = [bpool.tile([128, N], F32R, tag=f"m{b}", name=f"m{b}") for b in range(2)]
            uT = [bpool.tile([128, N], F32R, tag=f"uT{b}", name=f"uT{b}") for b in range(2)]
            ostage = stage
            out_r = r(out_d.rearrange("(t p) d -> p t d", p=128))
            def attention(h, half, with_v=False):
                qh = qT[h // 2][64 * (h % 2):64 * (h % 2) + 64, :]
                kh = kT[h // 2][64 * (h % 2):64 * (h % 2) + 64, :]
                dst = ectx[h // 2][64 * (h % 2):64 * (h % 2) + 64, :]
                hstart, hend = 1024 * half, 1024 * (half + 1)
                jb_max = 8 * (half + 1)
                ctx_q = [
                    ctxp.tile([65, 512], F32, tag="ctx", bufs=2, name="ctxq")
                    for _ in range(2)
                ]
                for jb in range(jb_max):
                    it0 = jb // 4

                    def col_start(it):
                        if it == it0:
                            return it * 512 + min(128 * (jb % 4), 256)
                        return it * 512

                    its = [t for t in range(max(it0, 2 * half), 2 * half + 2)]
                    if with_v and (half == 0 or jb >= 8):
                        emit_v(jb)
                    for it in its:
                        cst, cend = col_start(it), (it + 1) * 512
                        w = cend - cst
                        diag = it == it0
                        sc = scp.tile([128, 512], F32, tag="sc", name="sc")
                        nc.tensor.matmul(
                            sc[:, 0:w],
                            kh[:, jb * 128:(jb + 1) * 128],
                            qh[:, cst:cend],
                            start=True,
                            stop=True,
                        )
                        skip = 128 if (diag and jb % 4 == 3) else 0
                        e = wpool.tile([128, 512], F32R, tag="e", bufs=8, name="e")
                        nc.scalar.activation(
                            e[:, skip:w], sc[:, skip:w],
                            mybir.ActivationFunctionType.Exp,
                            scale=float(1.0 / np.sqrt(HD)),
                        )
                        if diag:
                            wd = 128 if (jb % 4) < 3 else 256
                            nc.gpsimd.affine_select(
                                e[:, 0:wd], e[:, 0:wd],
                                pattern=[[1, wd]],
                                compare_op=mybir.AluOpType.is_ge,
                                fill=0.0,
                                base=cst - 128 * jb,
                                channel_multiplier=-1,
                            )
                        last_jb = min(4 * it + 3, jb_max - 1)
                        cq = ctx_q[it - 2 * half]
                        qoff = it * 512
                        nc.tensor.matmul(
                            cq[0:65, cst - qoff:cend - qoff],
                            v_sb[jb][:, 65 * h:65 * h + 65],
                            e[:, 0:w],
                            start=(jb == 0),
                            stop=(jb == last_jb),
                            skip_group_check=True,
                        )
                        if jb == last_jb:
                            recip = wpool.tile(
                                [1, 512], F32, tag="recip", bufs=2, name="recip"
                            )
                            nc.vector.reciprocal(recip[:], cq[64:65, :])
                            rb = wpool.tile([64, 512], F32, tag="rb", bufs=2, name="rb")
                            nc.gpsimd.partition_broadcast(rb[:], recip[:])
                            nc.vector.tensor_tensor(
                                dst[:, it * 512:(it + 1) * 512],
                                cq[0:64, :],
                                rb[:],
                                op=mybir.AluOpType.mult,
                            )

            def tail(it):
                isl = slice(it * 512, (it + 1) * 512)
                for blk in range(2):
                    pt = mmp.tile([128, 512], F32, tag="mm", name="pt")
                    for c in range(2):
                        nc.tensor.matmul(
                            pt[:],
                            wo_sb[c][:, blk * 128:(blk + 1) * 128],
                            ectx[c][:, isl],
                            start=(c == 0),
                            stop=(c == 1),
                        )
                    nc.vector.tensor_scalar_add(m_sb[blk][:, isl], pt[:], bo_c[blk][:])
                for blk in range(2):
                    pt = mmp.tile([128, 512], F32, tag="mm", name="pt")
                    for c in range(4):
                        rhs = xT[c] if c < 2 else m_sb[c - 2]
                        nc.tensor.matmul(
                            pt[:],
                            wu_sb[c][:, blk * 128:(blk + 1) * 128],
                            rhs[:, isl],
                            start=(c == 0),
                            stop=(c == 3),
                        )
                    nc.vector.tensor_scalar(
                        uT[blk][:, isl], pt[:], bu_c[blk][:], 0.0,
                        op0=mybir.AluOpType.add, op1=mybir.AluOpType.max,
                    )
                for ib in range(it * 4, (it + 1) * 4):
                    for blk in range(2):
                        tp = scp.tile([128, 128], F32R, tag="sc", name="tp")
                        nc.tensor.transpose(
                            tp[:], uT[blk][:, ib * 128:(ib + 1) * 128], ident[:]
                        )
                        nc.scalar.copy(
                            ostage[:, ib * DM + blk * 128:ib * DM + (blk + 1) * 128],
                            tp[:],
                        )
                for g2 in range(4):
                    t0 = it * 4 + g2
                    nc.sync.dma_start(
                        out_r[:, t0:t0 + 1, :],
                        ostage[:, t0 * DM:(t0 + 1) * DM].rearrange(
                            "p (t d) -> p t d", d=DM
                        ),
                    )

            for h in range(H):
                if h == 2:
                    emit_qk(1)
                for half in range(2):
                    attention(h, half, with_v=(h == 0))
            for it in range(IT):
                tail(it)

    nc.compile()
    return nc


_STATE = {}


def _get_runner():
    if "run" in _STATE:
        return _STATE["run"]
    import jax
    from concourse.bass2jax import (
        _bass_exec_p,
        install_neuronx_cc_hook,
        partition_id_tensor,
    )
    from jax.sharding import Mesh, PartitionSpec
    from jax.experimental.shard_map import shard_map

    nc = build_program()
    install_neuronx_cc_hook()
    partition_name = nc.partition_id_tensor.name if nc.partition_id_tensor else None
    in_names, out_names, out_avals, zero_outs = [], [], [], []
    for alloc in nc.m.functions[0].allocations:
        if not isinstance(alloc, mybir.MemoryLocationSet):
            continue
        name = alloc.memorylocations[0].name
        if alloc.kind == "ExternalInput":
            if name != partition_name:
                in_names.append(name)
        elif alloc.kind == "ExternalOutput":
            shape = tuple(alloc.tensor_shape)
            dtype = mybir.dt.np(alloc.dtype)
            out_names.append(name)
            out_avals.append(jax.core.ShapedArray(shape, dtype))
            zero_outs.append(np.zeros(shape, dtype))
    n_params = len(in_names)
    all_in = in_names + out_names + ([partition_name] if partition_name else [])

    def _body(*args):
        operands = list(args)
        if partition_name is not None:
            operands.append(partition_id_tensor())
        return tuple(
            _bass_exec_p.bind(
                *operands,
                out_avals=tuple(out_avals),
                in_names=tuple(all_in),
                out_names=tuple(out_names),
                lowering_input_output_aliases=(),
                sim_require_finite=True,
                sim_require_nnan=True,
                nc=nc,
            )
        )

    devices = jax.devices()[:B]
    mesh = Mesh(np.asarray(devices), ("core",))
    specs = (PartitionSpec("core"),) * (n_params + len(out_names))
    jitted = jax.jit(
        shard_map(
            _body, mesh=mesh, in_specs=specs,
            out_specs=(PartitionSpec("core"),) * len(out_names), check_rep=False,
        ),
        keep_unused=True,
    )

    def run(in_maps):
        import jax as _jax

        concat_in = [
            np.concatenate([np.asarray(m[nm]) for m in in_maps], axis=0)
            for nm in in_names
        ]
        concat_zero = [
            np.zeros((B * z.shape[0], *z.shape[1:]), z.dtype) for z in zero_outs
        ]
        outs = jitted(*concat_in, *concat_zero)
        _jax.block_until_ready(outs)
        res = []
        o = np.asarray(outs[out_names.index("out")])
        per = o.shape[0] // B
        for c in range(B):
            res.append(o[c * per:(c + 1) * per])
        return res

    _STATE["run"] = run
    return run


def make_in_maps(node_features, Wq, bq, Wk, bk, Wv, bv, Wo, bo, Wu, bu):
    in_maps = []
    for c in range(B):
        in_maps.append(
            {
                "x": np.ascontiguousarray(node_features[c], dtype=np.float32),
                "wq": np.asarray(Wq, np.float32),
                "wk": np.asarray(Wk, np.float32),
                "wv": np.asarray(Wv, np.float32),
                "wo": np.asarray(Wo, np.float32),
                "wu": np.asarray(Wu, np.float32),
                "bq": np.asarray(bq, np.float32),
                "bk": np.asarray(bk, np.float32),
                "bv": np.asarray(bv, np.float32),
                "bo": np.asarray(bo, np.float32),
                "bu": np.asarray(bu, np.float32),
            }
        )
    return in_maps


def kernel(
    node_features, causal_mask, Wq, bq, Wk, bk, Wv, bv, Wo, bo, Wu, bu
):
    """Full-input entry point: shards batch across 8 cores internally."""
    del causal_mask  # guaranteed tril(ones); mask generated on-chip
    run = _get_runner()
    in_maps = make_in_maps(node_features, Wq, bq, Wk, bk, Wv, bv, Wo, bo, Wu, bu)
    outs = run(in_maps)
    return np.stack(outs, axis=0)
